# revision 5
# baseline (speedup 1.0000x reference)
"""Trainium2 Bass kernel for nn_Attention_35107062677619.

Dense transformer attention block (B=2, S=2048, D=4096, 32 Q heads / 8 KV
heads, head_dim 128, RoPE, causal mask) tensor-parallel over 8 NeuronCores.

Sharding: each core owns 4 Q heads + their shared KV head (GQA groups align
with cores), computes projections + RoPE + attention for those heads, then an
on-device AllGather collects the per-core attention outputs and each core
applies its 512-row slice of wo.  The host concatenates the 8 output-feature
slices.

Layout strategy (T = feature-major "transposed" layout [feat, tok]):
 - host feeds tile-contiguous xT blocks and pre-transposed bf16 weight shards
   so every matmul operand is a natural SBUF slice; the only on-device
   transpose is V (PE transpose through an identity).
 - Q/K rows are permuted per head into de-interleaved RoPE order (x0 block /
   x1 block) so RoPE becomes partition-shifted copies + multiplies with
   host-precomputed cos/sin tables (signs baked into the sin table).
 - scores are computed transposed (S_T[k, q]); softmax is max-free (scores
   are O(10) pre-mask), the causal mask is applied post-exp as a multiply by
   exp(mask) (exact for -inf/0 masks), fully-masked score tiles are skipped,
   and the k-reduction runs as accumulating M=1 ones-matmuls on the PE with
   the reciprocal broadcast done by gpsimd.partition_broadcast.
 - exp runs on 2-tile [128,1024] PSUM groups to amortize the ACT pipeline
   constant; attention internals are bf16 with fp32 PSUM accumulation.
 - one AllGather per (batch, 512-query tile) so the collectives fully overlap
   attention compute, and the wo phase consumes gather results per tile.
"""

import math
import os

import numpy as np
import ml_dtypes

B = 2
S = 2048
D = 4096
HD = 128
N_HEADS = 32
N_KV = 8
N_CORES = 8
NQH = N_HEADS // N_CORES  # 4 local Q heads
P = 128
SLAB = 512  # token tile (matmul free dim)
KH = D // P  # 32 hidden k-tiles
QKVD = NQH * HD + 2 * HD  # 768 projection output dims
F32 = np.float32
BF16 = ml_dtypes.bfloat16


def _build(nc_cores=N_CORES, s=S):
    """Build the SPMD Bass program (one program, data-parallel over cores)."""
    import concourse.masks as masks
    import concourse.mybir as mybir
    import concourse.tile as tile
    from concourse import bacc

    f32 = mybir.dt.float32
    f32r = mybir.dt.float32r
    bf16 = mybir.dt.bfloat16
    EXP = mybir.ActivationFunctionType.Exp

    tok = B * s
    nslab = tok // SLAB
    sslab = s // SLAB  # slabs per batch
    nqt = s // SLAB  # q-tiles (512) per batch
    nkt = s // P  # k-tiles (128) per batch
    dimt = QKVD // P  # 6 projection dim-tiles
    spk = SLAB // P  # 4  (128-tiles per 512 tile)

    nc = bacc.Bacc("TRN2", target_bir_lowering=False, debug=False,
                   num_devices=nc_cores)

    xT = nc.dram_tensor("xT", [KH * (tok // SLAB) * P, SLAB], bf16,
                    kind="ExternalInput")
    wqkvT = nc.dram_tensor("wqkvT", [D, QKVD], bf16,
                       kind="ExternalInput")
    woT = nc.dram_tensor("woT", [nc_cores * NQH * HD, SLAB], bf16,
                         kind="ExternalInput")
    cosq = nc.dram_tensor("cosq", [P, s], bf16, kind="ExternalInput")
    sinq = nc.dram_tensor("sinq", [P, s], bf16, kind="ExternalInput")
    expmaskT = nc.dram_tensor("expmaskT", [nqt * SLAB, SLAB], bf16,
                              kind="ExternalInput")
    outT = nc.dram_tensor("outT", [SLAB, tok], f32, kind="ExternalOutput")

    xT_r = xT.ap().rearrange("(o p) t -> o p t", p=P)  # [KH*nslab, 128, 512]
    wqkvT_r = wqkvT.ap().rearrange("(o p) q -> p o q", p=P)
    woT_r = woT.ap().rearrange("(o p) q -> p o q", p=P)
    expmaskT_r = expmaskT.ap().rearrange("(a p) q -> p a q", p=P)

    with tile.TileContext(nc) as tc:
        with (
            tc.tile_pool(name="persist", bufs=1) as persist,
            tc.tile_pool(name="dram", bufs=1, space="DRAM") as dram,
        ):
            cc_in = [dram.tile([NQH * HD, SLAB], bf16, tag=f"cc_in{i}",
                               name=f"cc_in{i}")
                     for i in range(B * nqt)]
            cc_out = [dram.tile([nc_cores * NQH * HD, SLAB], bf16,
                                tag=f"cc_out{i}", name=f"cc_out{i}")
                      for i in range(B * nqt)]
            cc_out_r = [t[:].rearrange("(o p) t -> p o t", p=P)
                        for t in cc_out]

            cos_sb = persist.tile([P, s], bf16, tag="cos")
            sin_sb = persist.tile([P, s], bf16, tag="sin")
            nc.sync.dma_start(cos_sb[:], cosq.ap())
            nc.sync.dma_start(sin_sb[:], sinq.ap())
            iden = persist.tile([P, P], bf16, tag="iden")
            masks.make_identity(nc, iden[:])
            ones_bf = persist.tile([P, 1], bf16, tag="onesbf")
            nc.vector.memset(ones_bf[:], 1.0)

            QT = persist.tile([P, NQH, tok], bf16, tag="QT")
            KT = persist.tile([P, tok], bf16, tag="KT")
            V = persist.tile([P, tok // P, HD], bf16, tag="V")
            emask_sb = persist.tile([P, nqt * spk, SLAB], bf16, tag="emask")
            nc.sync.dma_start(emask_sb[:], expmaskT_r)

            # ---- Phase A: fused QKV projection + RoPE (+ V transpose) ----
            with (
                nc.named_scope("phaseA"),
                tc.tile_pool(name="wqkv", bufs=1) as wpool,
                tc.tile_pool(name="xa", bufs=8) as xpool,
                tc.tile_pool(name="ropetmp", bufs=4) as rpool,
                tc.tile_pool(name="psA", bufs=6, space="PSUM") as psA,
                tc.tile_pool(name="psT", bufs=2, space="PSUM") as psT,
            ):
                wqkv_sb = wpool.tile([P, KH, QKVD], bf16, tag="wqkv")
                nc.sync.dma_start(wqkv_sb[:], wqkvT_r)

                for slab in range(nslab):
                    sr = (slab % sslab) * SLAB
                    t0 = slab * SLAB
                    psums = [psA.tile([P, SLAB], f32, tag="proj",
                                      name=f"proj_{slab}_{d}")
                             for d in range(dimt)]
                    for kb in range(KH):
                        xt = xpool.tile([P, SLAB], bf16, tag="x",
                                        name=f"x_{slab}_{kb}")
                        nc.sync.dma_start(xt[:], xT_r[kb * nslab + slab])
                        for d in range(dimt):
                            nc.tensor.matmul(
                                psums[d][:],
                                wqkv_sb[:, kb, d * P:(d + 1) * P],
                                xt[:],
                                start=(kb == 0), stop=(kb == KH - 1),
                            )
                    cs = cos_sb[:, sr:sr + SLAB]
                    sn = sin_sb[:, sr:sr + SLAB]
                    for d in range(NQH + 1):
                        dst = (QT[:, d, t0:t0 + SLAB] if d < NQH
                               else KT[:, t0:t0 + SLAB])
                        h = P // 2
                        # copy out of PSUM first (frees the bank for the
                        # next slab's matmuls), then all-bf16 SBUF rope
                        q_sb = rpool.tile([P, SLAB], bf16, tag="qsb",
                                          name=f"qsb_{slab}_{d}")
                        if d % 2 == 1:
                            nc.scalar.copy(q_sb[:], psums[d][:])
                        else:
                            nc.vector.tensor_copy(q_sb[:], psums[d][:])
                        tmp = rpool.tile([P, SLAB], bf16, tag="ropetmp",
                                         name=f"rt_{slab}_{d}")
                        nc.vector.tensor_copy(tmp[0:h, :], q_sb[h:P, :])
                        nc.vector.tensor_copy(tmp[h:P, :], q_sb[0:h, :])
                        nc.vector.tensor_mul(tmp[:], tmp[:], sn)
                        nc.vector.tensor_mul(dst, q_sb[:], cs)
                        nc.vector.tensor_add(dst, dst, tmp[:])
                    # V: copy + PE-transpose into natural [tok, hd] tiles
                    vtmp = rpool.tile([P, SLAB], bf16, tag="vtmp",
                                      name=f"vt_{slab}")
                    nc.vector.tensor_copy(vtmp[:], psums[NQH + 1][:])
                    for j in range(spk):
                        pst = psT.tile([P, P], bf16, tag="vt",
                                       name=f"vtp_{slab}_{j}")
                        nc.tensor.transpose(pst[:], vtmp[:, j * P:(j + 1) * P],
                                            iden[:])
                        nc.vector.tensor_copy(V[:, slab * spk + j, :], pst[:])

            # ---- Phase B: attention per (batch, local head) ----
            with (
                nc.named_scope("phaseB"),
                tc.tile_pool(name="es", bufs=16) as espool,
                tc.tile_pool(name="accp", bufs=4) as accpool,
                tc.tile_pool(name="osb", bufs=6) as opool,
                tc.tile_pool(name="psS", bufs=2, space="PSUM") as psS,
                tc.tile_pool(name="psAV", bufs=2, space="PSUM") as psAV,
                tc.tile_pool(name="psR", bufs=2, space="PSUM") as psR,
            ):
                for b in range(B):
                    for qt in range(nqt):
                        nkb = min((qt + 1) * spk, nkt)
                        q0 = b * s + qt * SLAB
                        for l in range(NQH):
                            pfx = f"{b}_{l}_{qt}"
                            av = psAV.tile([P, SLAB], f32, tag="av",
                                           name=f"av_{pfx}")
                            sm = psR.tile([1, SLAB], f32, tag="r",
                                          name=f"sm_{pfx}")
                            for g in range(nkb // 2):
                                kb0 = 2 * g
                                stg = psS.tile([P, 2 * SLAB], f32, tag="st",
                                               name=f"st_{pfx}_{g}")
                                for j in range(2):
                                    kb = kb0 + j
                                    nc.tensor.matmul(
                                        stg[:, j * SLAB:(j + 1) * SLAB],
                                        KT[:, b * s + kb * P:
                                           b * s + (kb + 1) * P],
                                        QT[:, l, q0:q0 + SLAB],
                                        start=True, stop=True,
                                    )
                                es = espool.tile([P, 2 * SLAB], bf16,
                                                 tag="es", name=f"es_{pfx}_{g}")
                                nc.scalar.activation(es[:], stg[:], EXP)
                                for j in range(2):
                                    kb = kb0 + j
                                    if kb >= nkb - spk:
                                        jj = kb - (nkb - spk)
                                        nc.vector.tensor_mul(
                                            es[:, j * SLAB:(j + 1) * SLAB],
                                            es[:, j * SLAB:(j + 1) * SLAB],
                                            emask_sb[:, qt * spk + jj, :])
                                for j in range(2):
                                    kb = kb0 + j
                                    nc.tensor.matmul(
                                        av[:], V[:, b * nkt + kb, :],
                                        es[:, j * SLAB:(j + 1) * SLAB],
                                        start=(kb == 0), stop=(kb == nkb - 1),
                                    )
                                    nc.tensor.matmul(
                                        sm[:], ones_bf[:, 0:1],
                                        es[:, j * SLAB:(j + 1) * SLAB],
                                        start=(kb == 0), stop=(kb == nkb - 1),
                                    )
                            o_u = opool.tile([P, SLAB], bf16, tag="ou",
                                             name=f"ou_{pfx}")
                            nc.vector.tensor_copy(o_u[:], av[:])
                            rs = accpool.tile([1, SLAB], f32, tag="rs",
                                              name=f"rs_{pfx}")
                            nc.vector.reciprocal(rs[:], sm[:])
                            rbs = accpool.tile([P, SLAB], f32, tag="rbs",
                                               name=f"rbs_{pfx}")
                            nc.gpsimd.partition_broadcast(rbs[:], rs[:])
                            o = opool.tile([P, SLAB], bf16, tag="o",
                                           name=f"o_{pfx}")
                            nc.vector.tensor_mul(o[:], o_u[:], rbs[:])
                            nc.sync.dma_start(
                                cc_in[b * nqt + qt][l * HD:(l + 1) * HD, :],
                                o[:])
                        # AllGather this (batch, q-tile) across cores
                        with nc.named_scope("gather"):
                            nc.gpsimd.collective_compute(
                                "AllGather",
                                mybir.AluOpType.bypass,
                                ins=[cc_in[b * nqt + qt].opt()],
                                outs=[cc_out[b * nqt + qt].opt()],
                                replica_groups=[list(range(nc_cores))],
                            )

            # ---- Phase C: output projection (this core's 512 features) ----
            with (
                nc.named_scope("phaseC"),
                tc.tile_pool(name="wo", bufs=1) as wopool,
                tc.tile_pool(name="g", bufs=2) as gpool,
                tc.tile_pool(name="oc", bufs=4) as ocpool,
                tc.tile_pool(name="psC", bufs=8, space="PSUM") as psC,
            ):
                nakt = (nc_cores * NQH * HD) // P
                wo_sb = wopool.tile([P, nakt, SLAB], bf16, tag="wo")
                nc.sync.dma_start(wo_sb[:], woT_r)
                for slab in range(nslab):
                    t0 = slab * SLAB
                    psums = [psC.tile([P, SLAB], f32, tag="wops",
                                      name=f"wops_{slab}_{od}")
                             for od in range(spk)]
                    g = gpool.tile([P, nakt, SLAB], bf16, tag="g",
                                   name=f"g_{slab}")
                    nc.sync.dma_start(g[:], cc_out_r[slab])
                    for od in range(spk):
                        for kb in range(nakt):
                            nc.tensor.matmul(
                                psums[od][:],
                                wo_sb[:, kb, od * P:(od + 1) * P],
                                g[:, kb, :],
                                start=(kb == 0), stop=(kb == nakt - 1),
                            )
                    for od in range(spk):
                        osb = ocpool.tile([P, SLAB], f32, tag="oc",
                                          name=f"oc_{slab}_{od}")
                        nc.vector.tensor_copy(osb[:], psums[od][:])
                        nc.sync.dma_start(
                            outT.ap()[od * P:(od + 1) * P, t0:t0 + SLAB],
                            osb[:])

    nc.compile()
    return nc


def _prep_inputs(x, wq, wk, wv, wo, freqs_cos, freqs_sin, mask,
                 nc_cores=N_CORES, s=S):
    """Host-side sharding + layout prep. Returns per-core input maps."""
    tok = B * s
    nqt = s // SLAB
    x = np.asarray(x, F32)
    nslab = tok // SLAB
    # tiled layout: block (kb, slab) = x[slab, :, kb, :].T contiguous
    xT = np.ascontiguousarray(
        x.reshape(nslab, SLAB, D // P, P).transpose(2, 0, 3, 1)
    ).astype(BF16).reshape(D // P * nslab * P, SLAB)

    # de-interleave permutation within a head: [x0_0..x0_63, x1_0..x1_63]
    perm = np.concatenate([np.arange(0, HD, 2), np.arange(1, HD, 2)])

    cos = np.asarray(freqs_cos, F32)  # [s, 64]
    sin = np.asarray(freqs_sin, F32)
    cosq = np.ascontiguousarray(
        np.concatenate([cos.T, cos.T], axis=0)).astype(BF16)
    # the shifted partner is multiplied by the DESTINATION row's sin entry:
    # o_top = x0*c - x1*s  -> top rows carry -sin
    # o_bot = x1*c + x0*s  -> bottom rows carry +sin
    sinq = np.ascontiguousarray(
        np.concatenate([-sin.T, sin.T], axis=0)).astype(BF16)

    m = np.asarray(mask, F32).reshape(s, s)
    blocks = []
    for qt in range(nqt):
        blk = m[qt * SLAB:(qt + 1) * SLAB, qt * SLAB:(qt + 1) * SLAB]
        blocks.append(np.exp(blk.T))  # [k, q]
    expmaskT = np.ascontiguousarray(
        np.concatenate(blocks, axis=0)).astype(BF16)

    scale = 1.0 / math.sqrt(HD)
    in_maps = []
    for c in range(nc_cores):
        wq_c = np.asarray(wq, F32)[c * NQH * HD:(c + 1) * NQH * HD]  # [512, D]
        wq_c = (wq_c.reshape(NQH, HD, D)[:, perm, :] * scale).reshape(
            NQH * HD, D)
        wk_c = np.asarray(wk, F32)[c * HD:(c + 1) * HD][perm, :]  # [128, D]
        wv_c = np.asarray(wv, F32)[c * HD:(c + 1) * HD]  # [128, D]
        wqkvT = np.ascontiguousarray(
            np.concatenate([wq_c, wk_c, wv_c], axis=0).T).astype(BF16)
        woT = np.ascontiguousarray(
            np.asarray(wo, F32)[c * SLAB:(c + 1) * SLAB].T).astype(BF16)
        in_maps.append({
            "xT": xT,
            "wqkvT": wqkvT,
            "woT": woT,
            "cosq": cosq,
            "sinq": sinq,
            "expmaskT": expmaskT,
        })
    return in_maps


_NC_CACHE = {}


def _get_nc(nc_cores=N_CORES, s=S):
    key = (nc_cores, s)
    if key not in _NC_CACHE:
        _NC_CACHE[key] = _build(nc_cores, s)
    return _NC_CACHE[key]


def _assemble(results, nc_cores=N_CORES, s=S):
    out = np.empty((B, s, nc_cores * SLAB), dtype=F32)
    for c in range(nc_cores):
        oT = results[c]["outT"]  # [512, tok]
        out[:, :, c * SLAB:(c + 1) * SLAB] = oT.T.reshape(B, s, SLAB)
    return out


def _run(inputs, trace=False, nc_cores=N_CORES, s=S):
    from concourse.bass_utils import run_bass_kernel_spmd

    nc = _get_nc(nc_cores, s)
    in_maps = _prep_inputs(**inputs, nc_cores=nc_cores, s=s)
    res = run_bass_kernel_spmd(nc, in_maps, core_ids=list(range(nc_cores)),
                               trace=trace)
    return _assemble(res.results, nc_cores, s), res


def kernel(x, wq, wk, wv, wo, freqs_cos, freqs_sin, mask):
    out, _ = _run(dict(x=x, wq=wq, wk=wk, wv=wv, wo=wo,
                       freqs_cos=freqs_cos, freqs_sin=freqs_sin, mask=mask),
                  trace=bool(int(os.environ.get("KERNEL_TRACE", "0"))))
    return out



# revision 6
# speedup vs baseline: 1.0125x; 1.0125x over previous
"""Trainium2 Bass kernel for nn_Attention_35107062677619.

Dense transformer attention block (B=2, S=2048, D=4096, 32 Q heads / 8 KV
heads, head_dim 128, RoPE, causal mask) tensor-parallel over 8 NeuronCores.

Sharding: each core owns 4 Q heads + their shared KV head (GQA groups align
with cores), computes projections + RoPE + attention for those heads, then an
on-device AllGather collects the per-core attention outputs and each core
applies its 512-row slice of wo.  The host concatenates the 8 output-feature
slices.

Layout strategy (T = feature-major "transposed" layout [feat, tok]):
 - host feeds tile-contiguous xT blocks and pre-transposed bf16 weight shards
   so every matmul operand is a natural SBUF slice; the only on-device
   transpose is V (PE transpose through an identity).
 - Q/K rows are permuted per head into de-interleaved RoPE order (x0 block /
   x1 block) so RoPE becomes partition-shifted copies + multiplies with
   host-precomputed cos/sin tables (signs baked into the sin table).
 - scores are computed transposed (S_T[k, q]); softmax is max-free (scores
   are O(10) pre-mask), causality is exploited at 128-column granularity
   (scores/exp/AV/rowsum restricted to the valid q-range per k-tile) with a
   single [128,128] exp-mask multiply on the exact diagonal sub-block, and
   the k-reduction runs as accumulating M=1 ones-matmuls on the PE with the
   reciprocal done by a fast custom-DVE op + gpsimd partition_broadcast.
 - phase B is software-pipelined: the scores matmul for item i+1 issues
   before the AV/rowsum matmuls of item i, so the scalar-engine exp of item
   i+1 overlaps the PE work of item i and the PE never waits on exp.
 - one AllGather per (batch, 512-query tile); the wo output projection is
   interleaved into the attention loop at tile granularity (lag 2) so the
   collectives overlap compute and only the last tile's gather + wo slab
   remain as tail.
"""

import math
import os

import numpy as np
import ml_dtypes

B = 2
S = 2048
D = 4096
HD = 128
N_HEADS = 32
N_KV = 8
N_CORES = 8
NQH = N_HEADS // N_CORES  # 4 local Q heads
P = 128
SLAB = 512  # token tile (matmul free dim)
KH = D // P  # 32 hidden k-tiles
QKVD = NQH * HD + 2 * HD  # 768 projection output dims
F32 = np.float32
BF16 = ml_dtypes.bfloat16


def _build(nc_cores=N_CORES, s=S):
    """Build the SPMD Bass program (one program, data-parallel over cores)."""
    import concourse.masks as masks
    import concourse.mybir as mybir
    import concourse.tile as tile
    from concourse import bacc

    f32 = mybir.dt.float32
    bf16 = mybir.dt.bfloat16
    EXP = mybir.ActivationFunctionType.Exp

    tok = B * s
    nslab = tok // SLAB
    sslab = s // SLAB  # slabs per batch
    nqt = s // SLAB  # q-tiles (512) per batch
    nkt = s // P  # k-tiles (128) per batch
    dimt = QKVD // P  # 6 projection dim-tiles
    spk = SLAB // P  # 4  (128-tiles per 512 tile)
    nakt = (nc_cores * NQH * HD) // P  # 32 wo contraction tiles

    nc = bacc.Bacc("TRN2", target_bir_lowering=False, debug=False,
                   num_devices=nc_cores)

    xT = nc.dram_tensor("xT", [KH * (tok // SLAB) * P, SLAB], bf16,
                    kind="ExternalInput")
    wqkvT = nc.dram_tensor("wqkvT", [D, QKVD], bf16,
                       kind="ExternalInput")
    woT = nc.dram_tensor("woT", [nc_cores * NQH * HD, SLAB], bf16,
                         kind="ExternalInput")
    cosq = nc.dram_tensor("cosq", [P, s], bf16, kind="ExternalInput")
    sinq = nc.dram_tensor("sinq", [P, s], bf16, kind="ExternalInput")
    trimaskT = nc.dram_tensor("trimaskT", [P, P], bf16, kind="ExternalInput")
    outT = nc.dram_tensor("outT", [SLAB, tok], f32, kind="ExternalOutput")

    xT_r = xT.ap().rearrange("(o p) t -> o p t", p=P)  # [KH*nslab, 128, 512]
    wqkvT_r = wqkvT.ap().rearrange("(o p) q -> p o q", p=P)
    woT_r = woT.ap().rearrange("(o p) q -> p o q", p=P)

    with tile.TileContext(nc) as tc:
        with (
            tc.tile_pool(name="persist", bufs=1) as persist,
            tc.tile_pool(name="dram", bufs=1, space="DRAM") as dram,
        ):
            cc_in = [dram.tile([NQH * HD, SLAB], bf16, tag=f"cc_in{i}",
                               name=f"cc_in{i}")
                     for i in range(B * nqt)]
            cc_out = [dram.tile([nc_cores * NQH * HD, SLAB], bf16,
                                tag=f"cc_out{i}", name=f"cc_out{i}")
                      for i in range(B * nqt)]
            cc_out_r = [t[:].rearrange("(o p) t -> p o t", p=P)
                        for t in cc_out]

            iden = persist.tile([P, P], bf16, tag="iden")
            masks.make_identity(nc, iden[:])
            ones_bf = persist.tile([P, 1], bf16, tag="onesbf")
            nc.vector.memset(ones_bf[:], 1.0)
            trimask_sb = persist.tile([P, P], bf16, tag="trimask")
            nc.sync.dma_start(trimask_sb[:], trimaskT.ap())

            QT = persist.tile([P, NQH, tok], bf16, tag="QT")
            KT = persist.tile([P, tok], bf16, tag="KT")
            V = persist.tile([P, tok // P, HD], bf16, tag="V")
            wo_sb = persist.tile([P, nakt, SLAB], bf16, tag="wo")
            nc.sync.dma_start(wo_sb[:], woT_r)

            # ---- Warmup: keep the PE busy early so HAM un-throttles before
            # the first real matmuls arrive.
            with (
                tc.tile_pool(name="warm", bufs=1) as warm,
                tc.tile_pool(name="psW", bufs=1, space="PSUM") as psW,
            ):
                wz = warm.tile([P, P], bf16, tag="wz")
                nc.vector.memset(wz[:], 0.0)
                wx = warm.tile([P, SLAB], bf16, tag="wx")
                nc.vector.memset(wx[:], 0.0)
                pw = psW.tile([P, SLAB], f32, tag="pw")
                for i in range(16):
                    nc.tensor.matmul(pw[:], wz[:], wx[:],
                                     start=(i == 0), stop=(i == 15))

            # ---- Phase A: fused QKV projection + RoPE (+ V transpose) ----
            with (
                nc.named_scope("phaseA"),
                tc.tile_pool(name="wqkv", bufs=1) as wpool,
                tc.tile_pool(name="xa", bufs=8) as xpool,
                tc.tile_pool(name="ropetmp", bufs=4) as rpool,
                tc.tile_pool(name="psA", bufs=6, space="PSUM") as psA,
                tc.tile_pool(name="psT", bufs=2, space="PSUM") as psT,
            ):
                cos_sb = wpool.tile([P, s], bf16, tag="cos")
                sin_sb = wpool.tile([P, s], bf16, tag="sin")
                nc.sync.dma_start(cos_sb[:], cosq.ap())
                nc.sync.dma_start(sin_sb[:], sinq.ap())
                wqkv_sb = wpool.tile([P, KH, QKVD], bf16, tag="wqkv")
                nc.sync.dma_start(wqkv_sb[:], wqkvT_r)

                for slab in range(nslab):
                    sr = (slab % sslab) * SLAB
                    t0 = slab * SLAB
                    psums = [psA.tile([P, SLAB], f32, tag="proj",
                                      name=f"proj_{slab}_{d}")
                             for d in range(dimt)]
                    for kb in range(KH):
                        xt = xpool.tile([P, SLAB], bf16, tag="x",
                                        name=f"x_{slab}_{kb}")
                        nc.sync.dma_start(xt[:], xT_r[kb * nslab + slab])
                        for d in range(dimt):
                            nc.tensor.matmul(
                                psums[d][:],
                                wqkv_sb[:, kb, d * P:(d + 1) * P],
                                xt[:],
                                start=(kb == 0), stop=(kb == KH - 1),
                            )
                    cs = cos_sb[:, sr:sr + SLAB]
                    sn = sin_sb[:, sr:sr + SLAB]
                    for d in range(NQH + 1):
                        dst = (QT[:, d, t0:t0 + SLAB] if d < NQH
                               else KT[:, t0:t0 + SLAB])
                        h = P // 2
                        # copy out of PSUM first (frees the bank for the
                        # next slab's matmuls), then all-bf16 SBUF rope
                        q_sb = rpool.tile([P, SLAB], bf16, tag="qsb",
                                          name=f"qsb_{slab}_{d}")
                        if d % 2 == 1:
                            nc.scalar.copy(q_sb[:], psums[d][:])
                        else:
                            nc.vector.tensor_copy(q_sb[:], psums[d][:])
                        tmp = rpool.tile([P, SLAB], bf16, tag="ropetmp",
                                         name=f"rt_{slab}_{d}")
                        nc.vector.tensor_copy(tmp[0:h, :], q_sb[h:P, :])
                        nc.vector.tensor_copy(tmp[h:P, :], q_sb[0:h, :])
                        nc.vector.tensor_mul(tmp[:], tmp[:], sn)
                        nc.vector.tensor_mul(dst, q_sb[:], cs)
                        nc.vector.tensor_add(dst, dst, tmp[:])
                    # V: copy + PE-transpose into natural [tok, hd] tiles
                    vtmp = rpool.tile([P, SLAB], bf16, tag="vtmp",
                                      name=f"vt_{slab}")
                    nc.vector.tensor_copy(vtmp[:], psums[NQH + 1][:])
                    for j in range(spk):
                        pst = psT.tile([P, P], bf16, tag="vt",
                                       name=f"vtp_{slab}_{j}")
                        nc.tensor.transpose(pst[:], vtmp[:, j * P:(j + 1) * P],
                                            iden[:])
                        nc.vector.tensor_copy(V[:, slab * spk + j, :], pst[:])

            # ---- Phase B+C: attention + interleaved output projection ----
            with (
                nc.named_scope("attn"),
                tc.tile_pool(name="es", bufs=8) as espool,
                tc.tile_pool(name="osb", bufs=6) as opool,
                tc.tile_pool(name="rsp", bufs=2) as rspool,
                tc.tile_pool(name="rbsp", bufs=2) as rbspool,
                tc.tile_pool(name="g", bufs=2) as gpool,
                tc.tile_pool(name="oc", bufs=4) as ocpool,
                tc.tile_pool(name="psS", bufs=4, space="PSUM") as psS,
                tc.tile_pool(name="psAV", bufs=2, space="PSUM") as psAV,
                tc.tile_pool(name="psR", bufs=2, space="PSUM") as psR,
            ):
                tiles = [(b, qt) for b in range(B)
                         for qt in reversed(range(nqt))]
                g_tiles = {}

                def emit_g_dma(b, qt):
                    di = b * nqt + qt
                    g = gpool.tile([P, nakt, SLAB], bf16, tag="g",
                                   name=f"g_{di}")
                    nc.gpsimd.dma_start(g[:], cc_out_r[di])
                    g_tiles[di] = g

                def emit_c_chunk(b, qt):
                    di = b * nqt + qt
                    t0 = di * SLAB
                    g = g_tiles.pop(di)
                    for od in range(spk):
                        ps = psAV.tile([P, SLAB], f32, tag="av",
                                       name=f"wops_{di}_{od}")
                        for kbb in range(nakt):
                            nc.tensor.matmul(
                                ps[:],
                                wo_sb[:, kbb, od * P:(od + 1) * P],
                                g[:, kbb, :],
                                start=(kbb == 0), stop=(kbb == nakt - 1),
                            )
                        osb = ocpool.tile([P, SLAB], f32, tag="oc",
                                          name=f"oc_{di}_{od}")
                        nc.scalar.copy(osb[:], ps[:])
                        nc.sync.dma_start(
                            outT.ap()[od * P:(od + 1) * P, t0:t0 + SLAB],
                            osb[:])

                for ti, (b, qt) in enumerate(tiles):
                    nkb = spk * (qt + 1)
                    dstart = nkb - spk  # first diagonal k-tile
                    q0 = b * s + qt * SLAB
                    di = b * nqt + qt
                    pfx = f"{b}_{qt}"
                    items = [(l, kb) for l in range(NQH)
                             for kb in range(nkb)]
                    stgs = {}

                    def emit_scores(l, kb):
                        j = kb - dstart
                        qlo = j * P if j >= 0 else 0
                        stg = psS.tile([P, SLAB], f32, tag="st",
                                       name=f"st_{pfx}_{l}_{kb}")
                        nc.tensor.matmul(
                            stg[:, qlo:SLAB],
                            KT[:, b * s + kb * P: b * s + (kb + 1) * P],
                            QT[:, l, q0 + qlo: q0 + SLAB],
                            start=True, stop=True,
                        )
                        stgs[(l, kb)] = (stg, qlo)

                    avs = {}
                    sms = {}
                    emit_scores(*items[0])
                    for idx, (l, kb) in enumerate(items):
                        stg, qlo = stgs.pop((l, kb))
                        es = espool.tile([P, SLAB], bf16, tag="es",
                                         name=f"es_{pfx}_{l}_{kb}")
                        nc.scalar.activation(es[:, qlo:SLAB],
                                             stg[:, qlo:SLAB], EXP)
                        if idx + 1 < len(items):
                            emit_scores(*items[idx + 1])
                        if kb >= dstart:
                            nc.vector.tensor_mul(es[:, qlo:qlo + P],
                                                 es[:, qlo:qlo + P],
                                                 trimask_sb[:])
                        if kb == 0:
                            avs[l] = psAV.tile([P, SLAB], f32, tag="av",
                                               name=f"av_{pfx}_{l}")
                            sms[l] = psR.tile([1, SLAB], f32, tag="r",
                                              name=f"sm_{pfx}_{l}")
                        av, sm = avs[l], sms[l]
                        nc.tensor.matmul(
                            av[:, qlo:SLAB], V[:, b * nkt + kb, :],
                            es[:, qlo:SLAB],
                            start=(kb == 0), stop=(kb == nkb - 1),
                            skip_group_check=True,
                        )
                        nc.tensor.matmul(
                            sm[:, qlo:SLAB], ones_bf[:, 0:1],
                            es[:, qlo:SLAB],
                            start=(kb == 0), stop=(kb == nkb - 1),
                            skip_group_check=True,
                        )
                        if kb == nkb - 1:
                            rs = rspool.tile([1, SLAB], f32, tag="rs",
                                             name=f"rs_{pfx}_{l}")
                            nc.vector.reciprocal_approx_fast(rs[:], sm[:])
                            rbs = rbspool.tile([P, SLAB], f32, tag="rbs",
                                               name=f"rbs_{pfx}_{l}")
                            nc.gpsimd.partition_broadcast(rbs[:], rs[:])
                            o = opool.tile([P, SLAB], bf16, tag="o",
                                           name=f"o_{pfx}_{l}")
                            nc.vector.tensor_mul(o[:], av[:], rbs[:])
                            nc.sync.dma_start(
                                cc_in[di][l * HD:(l + 1) * HD, :], o[:])
                    # AllGather this (batch, q-tile) across cores
                    with nc.named_scope("gather"):
                        nc.gpsimd.collective_compute(
                            "AllGather",
                            mybir.AluOpType.bypass,
                            ins=[cc_in[di].opt()],
                            outs=[cc_out[di].opt()],
                            replica_groups=[list(range(nc_cores))],
                        )
                    if ti >= 1:
                        emit_g_dma(*tiles[ti - 1])
                    if ti >= 2:
                        emit_c_chunk(*tiles[ti - 2])
                emit_g_dma(*tiles[-1])
                emit_c_chunk(*tiles[-2])
                emit_c_chunk(*tiles[-1])

    nc.compile()
    return nc


def _prep_inputs(x, wq, wk, wv, wo, freqs_cos, freqs_sin, mask,
                 nc_cores=N_CORES, s=S):
    """Host-side sharding + layout prep. Returns per-core input maps."""
    tok = B * s
    x = np.asarray(x, F32)
    nslab = tok // SLAB
    # tiled layout: block (kb, slab) = x[slab, :, kb, :].T contiguous
    xT = np.ascontiguousarray(
        x.reshape(nslab, SLAB, D // P, P).transpose(2, 0, 3, 1)
    ).astype(BF16).reshape(D // P * nslab * P, SLAB)

    # de-interleave permutation within a head: [x0_0..x0_63, x1_0..x1_63]
    perm = np.concatenate([np.arange(0, HD, 2), np.arange(1, HD, 2)])

    cos = np.asarray(freqs_cos, F32)  # [s, 64]
    sin = np.asarray(freqs_sin, F32)
    cosq = np.ascontiguousarray(
        np.concatenate([cos.T, cos.T], axis=0)).astype(BF16)
    # the shifted partner is multiplied by the DESTINATION row's sin entry:
    # o_top = x0*c - x1*s  -> top rows carry -sin
    # o_bot = x1*c + x0*s  -> bottom rows carry +sin
    sinq = np.ascontiguousarray(
        np.concatenate([-sin.T, sin.T], axis=0)).astype(BF16)

    # exact exp-mask for the [128,128] diagonal sub-block (identical for
    # every 128-aligned diagonal position of a causal mask)
    m = np.asarray(mask, F32).reshape(s, s)
    trimaskT = np.ascontiguousarray(np.exp(m[0:P, 0:P]).T).astype(BF16)

    scale = 1.0 / math.sqrt(HD)
    in_maps = []
    for c in range(nc_cores):
        wq_c = np.asarray(wq, F32)[c * NQH * HD:(c + 1) * NQH * HD]  # [512, D]
        wq_c = (wq_c.reshape(NQH, HD, D)[:, perm, :] * scale).reshape(
            NQH * HD, D)
        wk_c = np.asarray(wk, F32)[c * HD:(c + 1) * HD][perm, :]  # [128, D]
        wv_c = np.asarray(wv, F32)[c * HD:(c + 1) * HD]  # [128, D]
        wqkvT = np.ascontiguousarray(
            np.concatenate([wq_c, wk_c, wv_c], axis=0).T).astype(BF16)
        woT = np.ascontiguousarray(
            np.asarray(wo, F32)[c * SLAB:(c + 1) * SLAB].T).astype(BF16)
        in_maps.append({
            "xT": xT,
            "wqkvT": wqkvT,
            "woT": woT,
            "cosq": cosq,
            "sinq": sinq,
            "trimaskT": trimaskT,
        })
    return in_maps


_NC_CACHE = {}


def _get_nc(nc_cores=N_CORES, s=S):
    key = (nc_cores, s)
    if key not in _NC_CACHE:
        _NC_CACHE[key] = _build(nc_cores, s)
    return _NC_CACHE[key]


def _assemble(results, nc_cores=N_CORES, s=S):
    out = np.empty((B, s, nc_cores * SLAB), dtype=F32)
    for c in range(nc_cores):
        oT = results[c]["outT"]  # [512, tok]
        out[:, :, c * SLAB:(c + 1) * SLAB] = oT.T.reshape(B, s, SLAB)
    return out


def _run(inputs, trace=False, nc_cores=N_CORES, s=S):
    from concourse.bass_utils import run_bass_kernel_spmd

    nc = _get_nc(nc_cores, s)
    in_maps = _prep_inputs(**inputs, nc_cores=nc_cores, s=s)
    res = run_bass_kernel_spmd(nc, in_maps, core_ids=list(range(nc_cores)),
                               trace=trace)
    return _assemble(res.results, nc_cores, s), res


def kernel(x, wq, wk, wv, wo, freqs_cos, freqs_sin, mask):
    out, _ = _run(dict(x=x, wq=wq, wk=wk, wv=wv, wo=wo,
                       freqs_cos=freqs_cos, freqs_sin=freqs_sin, mask=mask),
                  trace=bool(int(os.environ.get("KERNEL_TRACE", "0"))))
    return out


# revision 12
# speedup vs baseline: 1.1601x; 1.1458x over previous
"""Trainium2 Bass kernel for nn_Attention_35107062677619.

Dense transformer attention block (B=2, S=2048, D=4096, 32 Q heads / 8 KV
heads, head_dim 128, RoPE, causal mask) tensor-parallel over 8 NeuronCores.

Sharding: each core owns 4 Q heads + their shared KV head (GQA groups align
with cores), computes projections + RoPE + attention for those heads, then an
on-device AllGather collects the per-core attention outputs and each core
applies its 512-row slice of wo.  The host concatenates the 8 output-feature
slices.

Layout strategy (T = feature-major "transposed" layout [feat, tok]):
 - host feeds tile-contiguous xT blocks and pre-transposed bf16 weight shards
   so every matmul operand is a natural SBUF slice; the only on-device
   transpose is V (PE transpose through an identity).
 - Q/K rows are permuted per head into de-interleaved RoPE order (x0 block /
   x1 block) so RoPE becomes partition-shifted copies + multiplies with
   host-precomputed cos/sin tables (signs baked into the sin table).
 - scores are computed transposed (S_T[k, q]); softmax is max-free (scores
   are O(10) pre-mask), causality is exploited at 128-column granularity
   (scores/exp/AV/rowsum restricted to the valid q-range per k-tile) with a
   single [128,128] exp-mask multiply on the exact diagonal sub-block, and
   the k-reduction runs as accumulating M=1 ones-matmuls on the PE with the
   reciprocal done by a fast custom-DVE op + gpsimd partition_broadcast.
 - phase B is software-pipelined: the scores matmul for item i+1 issues
   before the AV/rowsum matmuls of item i, so the scalar-engine exp of item
   i+1 overlaps the PE work of item i and the PE never waits on exp.
 - one AllGather per (batch, 512-query tile); the wo output projection is
   interleaved into the attention loop at tile granularity (lag 2) so the
   collectives overlap compute and only the last tile's gather + wo slab
   remain as tail.
"""

import math
import os

import numpy as np
import ml_dtypes

B = 2
S = 2048
D = 4096
HD = 128
N_HEADS = 32
N_KV = 8
N_CORES = 8
NQH = N_HEADS // N_CORES  # 4 local Q heads
P = 128
SLAB = 512  # token tile (matmul free dim)
KH = D // P  # 32 hidden k-tiles
QKVD = NQH * HD + 2 * HD  # 768 projection output dims
F32 = np.float32
BF16 = ml_dtypes.bfloat16


def _build(nc_cores=N_CORES, s=S):
    """Build the SPMD Bass program (one program, data-parallel over cores)."""
    import concourse.masks as masks
    import concourse.mybir as mybir
    import concourse.tile as tile
    from concourse import bacc

    f32 = mybir.dt.float32
    bf16 = mybir.dt.bfloat16
    EXP = mybir.ActivationFunctionType.Exp

    tok = B * s
    nslab = tok // SLAB
    sslab = s // SLAB  # slabs per batch
    nqt = s // SLAB  # q-tiles (512) per batch
    nkt = s // P  # k-tiles (128) per batch
    dimt = QKVD // P  # 6 projection dim-tiles
    spk = SLAB // P  # 4  (128-tiles per 512 tile)
    nakt = (nc_cores * NQH * HD) // P  # 32 wo contraction tiles

    nc = bacc.Bacc("TRN2", target_bir_lowering=False, debug=False,
                   num_devices=nc_cores)

    xT = nc.dram_tensor("xT", [KH * (tok // SLAB) * P, SLAB], bf16,
                    kind="ExternalInput")
    wqkvT = nc.dram_tensor("wqkvT", [D, QKVD], bf16,
                       kind="ExternalInput")
    woT = nc.dram_tensor("woT", [nc_cores * NQH * HD, SLAB], bf16,
                         kind="ExternalInput")
    cosq = nc.dram_tensor("cosq", [P, s], bf16, kind="ExternalInput")
    sinq = nc.dram_tensor("sinq", [P, s], bf16, kind="ExternalInput")
    trimaskT = nc.dram_tensor("trimaskT", [P, P], bf16, kind="ExternalInput")
    outT = nc.dram_tensor("outT", [SLAB, tok], f32, kind="ExternalOutput")

    xT_r = xT.ap().rearrange("(o p) t -> o p t", p=P)  # [KH*nslab, 128, 512]
    wqkvT_r = wqkvT.ap().rearrange("(o p) q -> p o q", p=P)
    woT_r = woT.ap().rearrange("(o p) q -> p o q", p=P)

    with tile.TileContext(nc) as tc:
        with (
            tc.tile_pool(name="persist", bufs=1) as persist,
            tc.tile_pool(name="dram", bufs=1, space="DRAM") as dram,
        ):
            cc_in = [dram.tile([NQH * HD, SLAB], bf16, tag=f"cc_in{i}",
                               name=f"cc_in{i}")
                     for i in range(B * nqt)]
            cc_out = [dram.tile([nc_cores * NQH * HD, SLAB], bf16,
                                tag=f"cc_out{i}", name=f"cc_out{i}",
                                addr_space="Shared")
                      for i in range(B * nqt)]
            cc_out_r = [t[:].rearrange("(o p) t -> p o t", p=P)
                        for t in cc_out]

            iden = persist.tile([P, P], bf16, tag="iden")
            masks.make_identity(nc, iden[:])
            ones_bf = persist.tile([P, 1], bf16, tag="onesbf")
            nc.vector.memset(ones_bf[:], 1.0)
            # setup DMAs ride side queues so phase A's x/wqkv stream (sync
            # queue) starts moving immediately
            trimask_sb = persist.tile([P, P], bf16, tag="trimask")
            nc.scalar.dma_start(trimask_sb[:], trimaskT.ap())

            QT = persist.tile([P, NQH, tok], bf16, tag="QT")
            KT = persist.tile([P, tok], bf16, tag="KT")
            V = persist.tile([P, tok // P, HD], bf16, tag="V")
            wo_sb = persist.tile([P, nakt, SLAB], bf16, tag="wo")
            nc.scalar.dma_start(wo_sb[:], woT_r)

            # ---- Phase A: fused QKV projection + RoPE (+ V transpose) ----
            with (
                nc.named_scope("phaseA"),
                tc.tile_pool(name="wqkv", bufs=1) as wpool,
                tc.tile_pool(name="xa", bufs=8) as xpool,
                tc.tile_pool(name="ropetmp", bufs=4) as rpool,
                tc.tile_pool(name="psA", bufs=7, space="PSUM") as psA,
                tc.tile_pool(name="psT", bufs=1, space="PSUM") as psT,
            ):
                cos_sb = wpool.tile([P, s], bf16, tag="cos")
                sin_sb = wpool.tile([P, s], bf16, tag="sin")
                nc.scalar.dma_start(cos_sb[:], cosq.ap())
                nc.scalar.dma_start(sin_sb[:], sinq.ap())
                # wqkv is DMAed per k-tile, interleaved with slab 0's x tiles
                # below, so the first matmul can issue ~1us in
                wqkv_sb = wpool.tile([P, KH, QKVD], bf16, tag="wqkv")

                for slab in range(nslab):
                    sr = (slab % sslab) * SLAB
                    t0 = slab * SLAB
                    psums = [psA.tile([P, SLAB], f32, tag="proj",
                                      name=f"proj_{slab}_{d}")
                             for d in range(dimt)]
                    for kb in range(KH):
                        if slab == 0:
                            nc.sync.dma_start(wqkv_sb[:, kb, :],
                                              wqkvT_r[:, kb, :])
                        xt = xpool.tile([P, SLAB], bf16, tag="x",
                                        name=f"x_{slab}_{kb}")
                        nc.sync.dma_start(xt[:], xT_r[kb * nslab + slab])
                        for d in range(dimt):
                            nc.tensor.matmul(
                                psums[d][:],
                                wqkv_sb[:, kb, d * P:(d + 1) * P],
                                xt[:],
                                start=(kb == 0), stop=(kb == KH - 1),
                            )
                    cs = cos_sb[:, sr:sr + SLAB]
                    sn = sin_sb[:, sr:sr + SLAB]
                    for d in range(NQH + 1):
                        dst = (QT[:, d, t0:t0 + SLAB] if d < NQH
                               else KT[:, t0:t0 + SLAB])
                        h = P // 2
                        # copy out of PSUM first (frees the bank for the
                        # next slab's matmuls), then all-bf16 SBUF rope
                        q_sb = rpool.tile([P, SLAB], bf16, tag="qsb",
                                          name=f"qsb_{slab}_{d}")
                        if d % 2 == 1:
                            nc.scalar.copy(q_sb[:], psums[d][:])
                        else:
                            nc.vector.tensor_copy(q_sb[:], psums[d][:])
                        tmp = rpool.tile([P, SLAB], bf16, tag="ropetmp",
                                         name=f"rt_{slab}_{d}")
                        nc.vector.tensor_copy(tmp[0:h, :], q_sb[h:P, :])
                        nc.vector.tensor_copy(tmp[h:P, :], q_sb[0:h, :])
                        nc.vector.tensor_mul(tmp[:], tmp[:], sn)
                        nc.vector.tensor_mul(dst, q_sb[:], cs)
                        nc.vector.tensor_add(dst, dst, tmp[:])
                    # V: copy + PE-transpose into natural [tok, hd] tiles
                    vtmp = rpool.tile([P, SLAB], bf16, tag="vtmp",
                                      name=f"vt_{slab}")
                    nc.vector.tensor_copy(vtmp[:], psums[NQH + 1][:])
                    for j in range(spk):
                        pst = psT.tile([P, P], bf16, tag="vt",
                                       name=f"vtp_{slab}_{j}")
                        nc.tensor.transpose(pst[:], vtmp[:, j * P:(j + 1) * P],
                                            iden[:])
                        nc.vector.tensor_copy(V[:, slab * spk + j, :], pst[:])

            # ---- Phase B+C: attention + interleaved output projection ----
            with (
                nc.named_scope("attn"),
                tc.tile_pool(name="es", bufs=8) as espool,
                tc.tile_pool(name="acc", bufs=3) as accpool,
                tc.tile_pool(name="osb", bufs=6) as opool,
                tc.tile_pool(name="rsp", bufs=2) as rspool,
                tc.tile_pool(name="rbsp", bufs=2) as rbspool,
                tc.tile_pool(name="g", bufs=2) as gpool,
                tc.tile_pool(name="oc", bufs=4) as ocpool,
                tc.tile_pool(name="psS", bufs=3, space="PSUM") as psS,
                tc.tile_pool(name="psAV", bufs=3, space="PSUM") as psAV,
                tc.tile_pool(name="psR", bufs=2, space="PSUM") as psR,
            ):
                tiles = [(b, qt) for b in range(B)
                         for qt in reversed(range(nqt))]
                g_tiles = {}

                def emit_g_dma(b, qt):
                    di = b * nqt + qt
                    g = gpool.tile([P, nakt, SLAB], bf16, tag="g",
                                   name=f"g_{di}")
                    nc.gpsimd.dma_start(g[:], cc_out_r[di])
                    g_tiles[di] = g

                def emit_c_chunk(b, qt):
                    di = b * nqt + qt
                    t0 = di * SLAB
                    g = g_tiles.pop(di)
                    for od in range(spk):
                        ps = psAV.tile([P, SLAB], f32, tag="av",
                                       name=f"wops_{di}_{od}")
                        for kbb in range(nakt):
                            nc.tensor.matmul(
                                ps[:],
                                wo_sb[:, kbb, od * P:(od + 1) * P],
                                g[:, kbb, :],
                                start=(kbb == 0), stop=(kbb == nakt - 1),
                            )
                        osb = ocpool.tile([P, SLAB], f32, tag="oc",
                                          name=f"oc_{di}_{od}")
                        nc.scalar.copy(osb[:], ps[:])
                        nc.sync.dma_start(
                            outT.ap()[od * P:(od + 1) * P, t0:t0 + SLAB],
                            osb[:])

                for ti, (b, qt) in enumerate(tiles):
                    nkb = spk * (qt + 1)
                    dstart = nkb - spk  # first diagonal k-tile
                    q0 = b * s + qt * SLAB
                    di = b * nqt + qt
                    pfx = f"{b}_{qt}"
                    items = [(l, kb) for l in range(NQH)
                             for kb in range(nkb)]
                    stgs = {}

                    def emit_scores(l, kb):
                        j = kb - dstart
                        qlo = j * P if j >= 0 else 0
                        stg = psS.tile([P, SLAB], f32, tag="st",
                                       name=f"st_{pfx}_{l}_{kb}")
                        nc.tensor.matmul(
                            stg[:, qlo:SLAB],
                            KT[:, b * s + kb * P: b * s + (kb + 1) * P],
                            QT[:, l, q0 + qlo: q0 + SLAB],
                            start=True, stop=True,
                        )
                        stgs[(l, kb)] = (stg, qlo)

                    avs = {}
                    accs = {}
                    emit_scores(*items[0])
                    for idx, (l, kb) in enumerate(items):
                        stg, qlo = stgs.pop((l, kb))
                        es = espool.tile([P, SLAB], bf16, tag="es",
                                         name=f"es_{pfx}_{l}_{kb}")
                        nc.scalar.activation(es[:, qlo:SLAB],
                                             stg[:, qlo:SLAB], EXP)
                        if idx + 1 < len(items):
                            emit_scores(*items[idx + 1])
                        if kb >= dstart:
                            nc.vector.tensor_mul(es[:, qlo:qlo + P],
                                                 es[:, qlo:qlo + P],
                                                 trimask_sb[:])
                        if kb == 0:
                            avs[l] = psAV.tile([P, SLAB], f32, tag="av",
                                               name=f"av_{pfx}_{l}")
                            accs[l] = accpool.tile([P, SLAB], bf16,
                                                   tag="acc",
                                                   name=f"acc_{pfx}_{l}")
                        av, acc = avs[l], accs[l]
                        nc.tensor.matmul(
                            av[:, qlo:SLAB], V[:, b * nkt + kb, :],
                            es[:, qlo:SLAB],
                            start=(kb == 0), stop=(kb == nkb - 1),
                            skip_group_check=True,
                        )
                        # k-partial rowsums accumulate on DVE (frees the PE
                        # from per-k-tile ones-matmuls)
                        if kb == 0:
                            nc.vector.tensor_copy(acc[:], es[:])
                        else:
                            nc.vector.tensor_add(acc[:, qlo:SLAB],
                                                 acc[:, qlo:SLAB],
                                                 es[:, qlo:SLAB])
                        if kb == nkb - 1:
                            sm = psR.tile([1, SLAB], f32, tag="r",
                                          name=f"sm_{pfx}_{l}")
                            nc.tensor.matmul(sm[:], ones_bf[:, 0:1], acc[:],
                                             start=True, stop=True)
                            rs = rspool.tile([1, SLAB], f32, tag="rs",
                                             name=f"rs_{pfx}_{l}")
                            nc.vector.reciprocal_approx_fast(rs[:], sm[:])
                            rbs = rbspool.tile([P, SLAB], f32, tag="rbs",
                                               name=f"rbs_{pfx}_{l}")
                            nc.gpsimd.partition_broadcast(rbs[:], rs[:])
                            o = opool.tile([P, SLAB], bf16, tag="o",
                                           name=f"o_{pfx}_{l}")
                            nc.vector.tensor_mul(o[:], av[:], rbs[:])
                            nc.sync.dma_start(
                                cc_in[di][l * HD:(l + 1) * HD, :], o[:])
                    # AllGather this (batch, q-tile) across cores
                    with nc.named_scope("gather"):
                        nc.gpsimd.collective_compute(
                            "AllGather",
                            mybir.AluOpType.bypass,
                            ins=[cc_in[di].opt()],
                            outs=[cc_out[di].opt()],
                            replica_groups=[list(range(nc_cores))],
                        )
                    if ti >= 2:
                        emit_g_dma(*tiles[ti - 2])
                    if ti >= 3:
                        emit_c_chunk(*tiles[ti - 3])
                emit_g_dma(*tiles[-2])
                emit_c_chunk(*tiles[-3])
                emit_g_dma(*tiles[-1])
                emit_c_chunk(*tiles[-2])
                emit_c_chunk(*tiles[-1])

    nc.compile()
    return nc


def _prep_inputs(x, wq, wk, wv, wo, freqs_cos, freqs_sin, mask,
                 nc_cores=N_CORES, s=S):
    """Host-side sharding + layout prep. Returns per-core input maps."""
    tok = B * s
    x = np.asarray(x, F32)
    nslab = tok // SLAB
    # tiled layout: block (kb, slab) = x[slab, :, kb, :].T contiguous
    xT = np.ascontiguousarray(
        x.reshape(nslab, SLAB, D // P, P).transpose(2, 0, 3, 1)
    ).astype(BF16).reshape(D // P * nslab * P, SLAB)

    # de-interleave permutation within a head: [x0_0..x0_63, x1_0..x1_63]
    perm = np.concatenate([np.arange(0, HD, 2), np.arange(1, HD, 2)])

    cos = np.asarray(freqs_cos, F32)  # [s, 64]
    sin = np.asarray(freqs_sin, F32)
    cosq = np.ascontiguousarray(
        np.concatenate([cos.T, cos.T], axis=0)).astype(BF16)
    # the shifted partner is multiplied by the DESTINATION row's sin entry:
    # o_top = x0*c - x1*s  -> top rows carry -sin
    # o_bot = x1*c + x0*s  -> bottom rows carry +sin
    sinq = np.ascontiguousarray(
        np.concatenate([-sin.T, sin.T], axis=0)).astype(BF16)

    # exact exp-mask for the [128,128] diagonal sub-block (identical for
    # every 128-aligned diagonal position of a causal mask)
    m = np.asarray(mask, F32).reshape(s, s)
    trimaskT = np.ascontiguousarray(np.exp(m[0:P, 0:P]).T).astype(BF16)

    scale = 1.0 / math.sqrt(HD)
    in_maps = []
    for c in range(nc_cores):
        wq_c = np.asarray(wq, F32)[c * NQH * HD:(c + 1) * NQH * HD]  # [512, D]
        wq_c = (wq_c.reshape(NQH, HD, D)[:, perm, :] * scale).reshape(
            NQH * HD, D)
        wk_c = np.asarray(wk, F32)[c * HD:(c + 1) * HD][perm, :]  # [128, D]
        wv_c = np.asarray(wv, F32)[c * HD:(c + 1) * HD]  # [128, D]
        wqkvT = np.ascontiguousarray(
            np.concatenate([wq_c, wk_c, wv_c], axis=0).T).astype(BF16)
        woT = np.ascontiguousarray(
            np.asarray(wo, F32)[c * SLAB:(c + 1) * SLAB].T).astype(BF16)
        in_maps.append({
            "xT": xT,
            "wqkvT": wqkvT,
            "woT": woT,
            "cosq": cosq,
            "sinq": sinq,
            "trimaskT": trimaskT,
        })
    return in_maps


_NC_CACHE = {}


def _get_nc(nc_cores=N_CORES, s=S):
    key = (nc_cores, s)
    if key not in _NC_CACHE:
        _NC_CACHE[key] = _build(nc_cores, s)
    return _NC_CACHE[key]


def _assemble(results, nc_cores=N_CORES, s=S):
    out = np.empty((B, s, nc_cores * SLAB), dtype=F32)
    for c in range(nc_cores):
        oT = results[c]["outT"]  # [512, tok]
        out[:, :, c * SLAB:(c + 1) * SLAB] = oT.T.reshape(B, s, SLAB)
    return out


def _run(inputs, trace=False, nc_cores=N_CORES, s=S):
    from concourse.bass_utils import run_bass_kernel_spmd

    nc = _get_nc(nc_cores, s)
    in_maps = _prep_inputs(**inputs, nc_cores=nc_cores, s=s)
    res = run_bass_kernel_spmd(nc, in_maps, core_ids=list(range(nc_cores)),
                               trace=trace)
    return _assemble(res.results, nc_cores, s), res


def kernel(x, wq, wk, wv, wo, freqs_cos, freqs_sin, mask):
    out, _ = _run(dict(x=x, wq=wq, wk=wk, wv=wv, wo=wo,
                       freqs_cos=freqs_cos, freqs_sin=freqs_sin, mask=mask),
                  trace=bool(int(os.environ.get("KERNEL_TRACE", "0"))))
    return out


# revision 13
# speedup vs baseline: 1.1680x; 1.0068x over previous
"""Trainium2 Bass kernel for nn_Attention_35107062677619.

Dense transformer attention block (B=2, S=2048, D=4096, 32 Q heads / 8 KV
heads, head_dim 128, RoPE, causal mask) tensor-parallel over 8 NeuronCores.

Sharding: each core owns 4 Q heads + their shared KV head (GQA groups align
with cores), computes projections + RoPE + attention for those heads, then an
on-device AllGather collects the per-core attention outputs and each core
applies its 512-row slice of wo.  The host concatenates the 8 output-feature
slices.

Layout strategy (T = feature-major "transposed" layout [feat, tok]):
 - host feeds tile-contiguous xT blocks and pre-transposed bf16 weight shards
   so every matmul operand is a natural SBUF slice; the only on-device
   transpose is V (PE transpose through an identity).
 - Q/K rows are permuted per head into de-interleaved RoPE order (x0 block /
   x1 block) so RoPE becomes partition-shifted copies + multiplies with
   host-precomputed cos/sin tables (signs baked into the sin table).
 - scores are computed transposed (S_T[k, q]); softmax is max-free (scores
   are O(10) pre-mask), causality is exploited at 128-column granularity
   (scores/exp/AV/rowsum restricted to the valid q-range per k-tile) with a
   single [128,128] exp-mask multiply on the exact diagonal sub-block, and
   the k-reduction runs as accumulating M=1 ones-matmuls on the PE with the
   reciprocal done by a fast custom-DVE op + gpsimd partition_broadcast.
 - phase B is software-pipelined: the scores matmul for item i+1 issues
   before the AV/rowsum matmuls of item i, so the scalar-engine exp of item
   i+1 overlaps the PE work of item i and the PE never waits on exp.
 - one AllGather per (batch, 512-query tile); the wo output projection is
   interleaved into the attention loop at tile granularity (lag 2) so the
   collectives overlap compute and only the last tile's gather + wo slab
   remain as tail.
"""

import math
import os

import numpy as np
import ml_dtypes

B = 2
S = 2048
D = 4096
HD = 128
N_HEADS = 32
N_KV = 8
N_CORES = 8
NQH = N_HEADS // N_CORES  # 4 local Q heads
P = 128
SLAB = 512  # token tile (matmul free dim)
KH = D // P  # 32 hidden k-tiles
QKVD = NQH * HD + 2 * HD  # 768 projection output dims
F32 = np.float32
BF16 = ml_dtypes.bfloat16


def _build(nc_cores=N_CORES, s=S):
    """Build the SPMD Bass program (one program, data-parallel over cores)."""
    import concourse.masks as masks
    import concourse.mybir as mybir
    import concourse.tile as tile
    from concourse import bacc

    f32 = mybir.dt.float32
    bf16 = mybir.dt.bfloat16
    EXP = mybir.ActivationFunctionType.Exp

    tok = B * s
    nslab = tok // SLAB
    sslab = s // SLAB  # slabs per batch
    nqt = s // SLAB  # q-tiles (512) per batch
    nkt = s // P  # k-tiles (128) per batch
    dimt = QKVD // P  # 6 projection dim-tiles
    spk = SLAB // P  # 4  (128-tiles per 512 tile)
    nakt = (nc_cores * NQH * HD) // P  # 32 wo contraction tiles

    nc = bacc.Bacc("TRN2", target_bir_lowering=False, debug=False,
                   num_devices=nc_cores)

    xT = nc.dram_tensor("xT", [KH * (tok // SLAB) * P, SLAB], bf16,
                    kind="ExternalInput")
    wqkvT = nc.dram_tensor("wqkvT", [D, QKVD], bf16,
                       kind="ExternalInput")
    woT = nc.dram_tensor("woT", [nc_cores * NQH * HD, SLAB], bf16,
                         kind="ExternalInput")
    cosq = nc.dram_tensor("cosq", [P, s], bf16, kind="ExternalInput")
    sinq = nc.dram_tensor("sinq", [P, s], bf16, kind="ExternalInput")
    trimaskT = nc.dram_tensor("trimaskT", [P, P], bf16, kind="ExternalInput")
    outT = nc.dram_tensor("outT", [SLAB, tok], f32, kind="ExternalOutput")

    xT_r = xT.ap().rearrange("(o p) t -> o p t", p=P)  # [KH*nslab, 128, 512]
    wqkvT_r = wqkvT.ap().rearrange("(o p) q -> p o q", p=P)
    woT_r = woT.ap().rearrange("(o p) q -> p o q", p=P)

    with tile.TileContext(nc) as tc:
        with (
            tc.tile_pool(name="persist", bufs=1) as persist,
            tc.tile_pool(name="dram", bufs=1, space="DRAM") as dram,
        ):
            cc_in = [dram.tile([NQH * HD, SLAB], bf16, tag=f"cc_in{i}",
                               name=f"cc_in{i}")
                     for i in range(B * nqt)]
            cc_out = [dram.tile([nc_cores * NQH * HD, SLAB], bf16,
                                tag=f"cc_out{i}", name=f"cc_out{i}",
                                addr_space="Shared")
                      for i in range(B * nqt)]
            cc_out_r = [t[:].rearrange("(o p) t -> p o t", p=P)
                        for t in cc_out]

            iden = persist.tile([P, P], bf16, tag="iden")
            masks.make_identity(nc, iden[:])
            ones_bf = persist.tile([P, 1], bf16, tag="onesbf")
            nc.vector.memset(ones_bf[:], 1.0)
            # setup DMAs ride side queues so phase A's x/wqkv stream (sync
            # queue) starts moving immediately
            trimask_sb = persist.tile([P, P], bf16, tag="trimask")
            nc.scalar.dma_start(trimask_sb[:], trimaskT.ap())

            QT = persist.tile([P, NQH, tok], bf16, tag="QT")
            KT = persist.tile([P, tok], bf16, tag="KT")
            V = persist.tile([P, tok // P, HD], bf16, tag="V")
            wo_sb = persist.tile([P, nakt, SLAB], bf16, tag="wo")
            nc.scalar.dma_start(wo_sb[:], woT_r)

            # ---- Phase A: fused QKV projection + RoPE (+ V transpose) ----
            with (
                nc.named_scope("phaseA"),
                tc.tile_pool(name="wqkv", bufs=1) as wpool,
                tc.tile_pool(name="xa", bufs=8) as xpool,
                tc.tile_pool(name="ropetmp", bufs=4) as rpool,
                tc.tile_pool(name="psA", bufs=7, space="PSUM") as psA,
                tc.tile_pool(name="psT", bufs=1, space="PSUM") as psT,
            ):
                cos_sb = wpool.tile([P, s], bf16, tag="cos")
                sin_sb = wpool.tile([P, s], bf16, tag="sin")
                nc.scalar.dma_start(cos_sb[:], cosq.ap())
                nc.scalar.dma_start(sin_sb[:], sinq.ap())
                # wqkv is DMAed per k-tile, interleaved with slab 0's x tiles
                # below, so the first matmul can issue ~1us in
                wqkv_sb = wpool.tile([P, KH, QKVD], bf16, tag="wqkv")

                for slab in range(nslab):
                    sr = (slab % sslab) * SLAB
                    t0 = slab * SLAB
                    psums = [psA.tile([P, SLAB], f32, tag="proj",
                                      name=f"proj_{slab}_{d}")
                             for d in range(dimt)]
                    for kb in range(KH):
                        if slab == 0:
                            nc.sync.dma_start(wqkv_sb[:, kb, :],
                                              wqkvT_r[:, kb, :])
                        xt = xpool.tile([P, SLAB], bf16, tag="x",
                                        name=f"x_{slab}_{kb}")
                        nc.sync.dma_start(xt[:], xT_r[kb * nslab + slab])
                        for d in range(dimt):
                            nc.tensor.matmul(
                                psums[d][:],
                                wqkv_sb[:, kb, d * P:(d + 1) * P],
                                xt[:],
                                start=(kb == 0), stop=(kb == KH - 1),
                            )
                    cs = cos_sb[:, sr:sr + SLAB]
                    sn = sin_sb[:, sr:sr + SLAB]
                    for d in range(NQH + 1):
                        dst = (QT[:, d, t0:t0 + SLAB] if d < NQH
                               else KT[:, t0:t0 + SLAB])
                        h = P // 2
                        # copy out of PSUM first (frees the bank for the
                        # next slab's matmuls), then all-bf16 SBUF rope
                        q_sb = rpool.tile([P, SLAB], bf16, tag="qsb",
                                          name=f"qsb_{slab}_{d}")
                        if d % 2 == 1:
                            nc.scalar.copy(q_sb[:], psums[d][:])
                        else:
                            nc.vector.tensor_copy(q_sb[:], psums[d][:])
                        tmp = rpool.tile([P, SLAB], bf16, tag="ropetmp",
                                         name=f"rt_{slab}_{d}")
                        nc.vector.tensor_copy(tmp[0:h, :], q_sb[h:P, :])
                        nc.vector.tensor_copy(tmp[h:P, :], q_sb[0:h, :])
                        nc.vector.tensor_mul(tmp[:], tmp[:], sn)
                        nc.vector.tensor_mul(dst, q_sb[:], cs)
                        nc.vector.tensor_add(dst, dst, tmp[:])
                    # V: copy + PE-transpose into natural [tok, hd] tiles
                    vtmp = rpool.tile([P, SLAB], bf16, tag="vtmp",
                                      name=f"vt_{slab}")
                    nc.vector.tensor_copy(vtmp[:], psums[NQH + 1][:])
                    for j in range(spk):
                        pst = psT.tile([P, P], bf16, tag="vt",
                                       name=f"vtp_{slab}_{j}")
                        nc.tensor.transpose(pst[:], vtmp[:, j * P:(j + 1) * P],
                                            iden[:])
                        nc.vector.tensor_copy(V[:, slab * spk + j, :], pst[:])

            # ---- Phase B+C: attention + interleaved output projection ----
            with (
                nc.named_scope("attn"),
                tc.tile_pool(name="es", bufs=8) as espool,
                tc.tile_pool(name="acc", bufs=3) as accpool,
                tc.tile_pool(name="osb", bufs=6) as opool,
                tc.tile_pool(name="rsp", bufs=2) as rspool,
                tc.tile_pool(name="rbsp", bufs=2) as rbspool,
                tc.tile_pool(name="g", bufs=2) as gpool,
                tc.tile_pool(name="oc", bufs=4) as ocpool,
                tc.tile_pool(name="psS", bufs=3, space="PSUM") as psS,
                tc.tile_pool(name="psAV", bufs=3, space="PSUM") as psAV,
                tc.tile_pool(name="psR", bufs=2, space="PSUM") as psR,
            ):
                tiles = [(b, qt) for b in range(B)
                         for qt in reversed(range(nqt))]
                g_tiles = {}

                def emit_g_dma(b, qt):
                    di = b * nqt + qt
                    g = gpool.tile([P, nakt, SLAB], bf16, tag="g",
                                   name=f"g_{di}")
                    nc.gpsimd.dma_start(g[:], cc_out_r[di])
                    g_tiles[di] = g

                def emit_c_chunk(b, qt):
                    di = b * nqt + qt
                    t0 = di * SLAB
                    g = g_tiles.pop(di)
                    for od in range(spk):
                        ps = psAV.tile([P, SLAB], f32, tag="av",
                                       name=f"wops_{di}_{od}")
                        for kbb in range(nakt):
                            nc.tensor.matmul(
                                ps[:],
                                wo_sb[:, kbb, od * P:(od + 1) * P],
                                g[:, kbb, :],
                                start=(kbb == 0), stop=(kbb == nakt - 1),
                            )
                        osb = ocpool.tile([P, SLAB], f32, tag="oc",
                                          name=f"oc_{di}_{od}")
                        nc.scalar.copy(osb[:], ps[:])
                        nc.sync.dma_start(
                            outT.ap()[od * P:(od + 1) * P, t0:t0 + SLAB],
                            osb[:])

                for ti, (b, qt) in enumerate(tiles):
                    nkb = spk * (qt + 1)
                    dstart = nkb - spk  # first diagonal k-tile
                    q0 = b * s + qt * SLAB
                    di = b * nqt + qt
                    pfx = f"{b}_{qt}"
                    items = [(l, kb) for l in range(NQH)
                             for kb in range(nkb)]
                    stgs = {}

                    def emit_scores(l, kb):
                        j = kb - dstart
                        qlo = j * P if j >= 0 else 0
                        stg = psS.tile([P, SLAB], f32, tag="st",
                                       name=f"st_{pfx}_{l}_{kb}")
                        nc.tensor.matmul(
                            stg[:, qlo:SLAB],
                            KT[:, b * s + kb * P: b * s + (kb + 1) * P],
                            QT[:, l, q0 + qlo: q0 + SLAB],
                            start=True, stop=True,
                        )
                        stgs[(l, kb)] = (stg, qlo)

                    avs = {}
                    accs = {}
                    emit_scores(*items[0])
                    for idx, (l, kb) in enumerate(items):
                        stg, qlo = stgs.pop((l, kb))
                        es = espool.tile([P, SLAB], bf16, tag="es",
                                         name=f"es_{pfx}_{l}_{kb}")
                        nc.scalar.activation(es[:, qlo:SLAB],
                                             stg[:, qlo:SLAB], EXP)
                        if idx + 1 < len(items):
                            emit_scores(*items[idx + 1])
                        if kb >= dstart:
                            nc.vector.tensor_mul(es[:, qlo:qlo + P],
                                                 es[:, qlo:qlo + P],
                                                 trimask_sb[:])
                        if kb == 0:
                            avs[l] = psAV.tile([P, SLAB], f32, tag="av",
                                               name=f"av_{pfx}_{l}")
                            accs[l] = accpool.tile([P, SLAB], bf16,
                                                   tag="acc",
                                                   name=f"acc_{pfx}_{l}")
                        av, acc = avs[l], accs[l]
                        nc.tensor.matmul(
                            av[:, qlo:SLAB], V[:, b * nkt + kb, :],
                            es[:, qlo:SLAB],
                            start=(kb == 0), stop=(kb == nkb - 1),
                            skip_group_check=True,
                        )
                        # k-partial rowsums accumulate on DVE (frees the PE
                        # from per-k-tile ones-matmuls)
                        if kb == 0:
                            nc.vector.tensor_copy(acc[:], es[:])
                        else:
                            nc.vector.tensor_add(acc[:, qlo:SLAB],
                                                 acc[:, qlo:SLAB],
                                                 es[:, qlo:SLAB])
                        if kb == nkb - 1:
                            sm = psR.tile([1, SLAB], f32, tag="r",
                                          name=f"sm_{pfx}_{l}")
                            nc.tensor.matmul(sm[:], ones_bf[:, 0:1], acc[:],
                                             start=True, stop=True)
                            rs = rspool.tile([1, SLAB], f32, tag="rs",
                                             name=f"rs_{pfx}_{l}")
                            nc.vector.reciprocal_approx_fast(rs[:], sm[:])
                            rbs = rbspool.tile([P, SLAB], f32, tag="rbs",
                                               name=f"rbs_{pfx}_{l}")
                            nc.gpsimd.partition_broadcast(rbs[:], rs[:])
                            o = opool.tile([P, SLAB], bf16, tag="o",
                                           name=f"o_{pfx}_{l}")
                            nc.vector.tensor_mul(o[:], av[:], rbs[:])
                            nc.sync.dma_start(
                                cc_in[di][l * HD:(l + 1) * HD, :], o[:])
                    # AllGather this (batch, q-tile) across cores
                    with nc.named_scope("gather"):
                        nc.gpsimd.collective_compute(
                            "AllGather",
                            mybir.AluOpType.bypass,
                            ins=[cc_in[di].opt()],
                            outs=[cc_out[di].opt()],
                            replica_groups=[list(range(nc_cores))],
                        )
                    if ti >= 3:
                        emit_g_dma(*tiles[ti - 3])
                    if ti >= 4:
                        emit_c_chunk(*tiles[ti - 4])
                emit_g_dma(*tiles[-3])
                emit_c_chunk(*tiles[-4])
                emit_g_dma(*tiles[-2])
                emit_c_chunk(*tiles[-3])
                emit_g_dma(*tiles[-1])
                emit_c_chunk(*tiles[-2])
                emit_c_chunk(*tiles[-1])

    nc.compile()
    return nc


def _prep_inputs(x, wq, wk, wv, wo, freqs_cos, freqs_sin, mask,
                 nc_cores=N_CORES, s=S):
    """Host-side sharding + layout prep. Returns per-core input maps."""
    tok = B * s
    x = np.asarray(x, F32)
    nslab = tok // SLAB
    # tiled layout: block (kb, slab) = x[slab, :, kb, :].T contiguous
    xT = np.ascontiguousarray(
        x.reshape(nslab, SLAB, D // P, P).transpose(2, 0, 3, 1)
    ).astype(BF16).reshape(D // P * nslab * P, SLAB)

    # de-interleave permutation within a head: [x0_0..x0_63, x1_0..x1_63]
    perm = np.concatenate([np.arange(0, HD, 2), np.arange(1, HD, 2)])

    cos = np.asarray(freqs_cos, F32)  # [s, 64]
    sin = np.asarray(freqs_sin, F32)
    cosq = np.ascontiguousarray(
        np.concatenate([cos.T, cos.T], axis=0)).astype(BF16)
    # the shifted partner is multiplied by the DESTINATION row's sin entry:
    # o_top = x0*c - x1*s  -> top rows carry -sin
    # o_bot = x1*c + x0*s  -> bottom rows carry +sin
    sinq = np.ascontiguousarray(
        np.concatenate([-sin.T, sin.T], axis=0)).astype(BF16)

    # exact exp-mask for the [128,128] diagonal sub-block (identical for
    # every 128-aligned diagonal position of a causal mask)
    m = np.asarray(mask, F32).reshape(s, s)
    trimaskT = np.ascontiguousarray(np.exp(m[0:P, 0:P]).T).astype(BF16)

    scale = 1.0 / math.sqrt(HD)
    in_maps = []
    for c in range(nc_cores):
        wq_c = np.asarray(wq, F32)[c * NQH * HD:(c + 1) * NQH * HD]  # [512, D]
        wq_c = (wq_c.reshape(NQH, HD, D)[:, perm, :] * scale).reshape(
            NQH * HD, D)
        wk_c = np.asarray(wk, F32)[c * HD:(c + 1) * HD][perm, :]  # [128, D]
        wv_c = np.asarray(wv, F32)[c * HD:(c + 1) * HD]  # [128, D]
        wqkvT = np.ascontiguousarray(
            np.concatenate([wq_c, wk_c, wv_c], axis=0).T).astype(BF16)
        woT = np.ascontiguousarray(
            np.asarray(wo, F32)[c * SLAB:(c + 1) * SLAB].T).astype(BF16)
        in_maps.append({
            "xT": xT,
            "wqkvT": wqkvT,
            "woT": woT,
            "cosq": cosq,
            "sinq": sinq,
            "trimaskT": trimaskT,
        })
    return in_maps


_NC_CACHE = {}


def _get_nc(nc_cores=N_CORES, s=S):
    key = (nc_cores, s)
    if key not in _NC_CACHE:
        _NC_CACHE[key] = _build(nc_cores, s)
    return _NC_CACHE[key]


def _assemble(results, nc_cores=N_CORES, s=S):
    out = np.empty((B, s, nc_cores * SLAB), dtype=F32)
    for c in range(nc_cores):
        oT = results[c]["outT"]  # [512, tok]
        out[:, :, c * SLAB:(c + 1) * SLAB] = oT.T.reshape(B, s, SLAB)
    return out


def _run(inputs, trace=False, nc_cores=N_CORES, s=S):
    from concourse.bass_utils import run_bass_kernel_spmd

    nc = _get_nc(nc_cores, s)
    in_maps = _prep_inputs(**inputs, nc_cores=nc_cores, s=s)
    res = run_bass_kernel_spmd(nc, in_maps, core_ids=list(range(nc_cores)),
                               trace=trace)
    return _assemble(res.results, nc_cores, s), res


def kernel(x, wq, wk, wv, wo, freqs_cos, freqs_sin, mask):
    out, _ = _run(dict(x=x, wq=wq, wk=wk, wv=wv, wo=wo,
                       freqs_cos=freqs_cos, freqs_sin=freqs_sin, mask=mask),
                  trace=bool(int(os.environ.get("KERNEL_TRACE", "0"))))
    return out
